# revision 6
# baseline (speedup 1.0000x reference)
"""Multi-head attention (B=2, N=2048, EMB=1024, 16 heads) on 8 TRN2 NeuronCores.

Sharding: data-parallel over batch (2) x tensor-parallel over heads (4 groups of
4 heads).  Core c handles batch c//4 and heads [4*(c%4), 4*(c%4)+4).  Each core:
  - projects its batch's q/k/v against the column slice of Wq/Wk/Wv for its heads
    (activations pre-transposed on host to [EMB, N] so features sit on SBUF
    partitions),
  - runs attention for its 4 heads in S^T orientation ([k_tokens, q_tokens]):
    softmax row-sums come for free from a ones-column appended to V in the
    P@V matmul; exp runs on the scalar engine straight out of PSUM,
  - applies the row-parallel slice of Wo, producing a partial [EMB, N] output,
  - emits the head-summed prob maps (transposed) for the total_attention_weights
    diagnostic as two bf16 partials.
Host sums partial outputs across the 4 cores of each batch (+ bo), and sums the
tw partials of batch-0 cores.  All device compute is bf16 with fp32 PSUM
accumulation.
"""
import sys

sys.path.insert(0, "/opt/trn_rl_repo")

from contextlib import ExitStack

import numpy as np
import ml_dtypes

import concourse.bass as bass
import concourse.bacc as bacc
import concourse.tile as tile
from concourse import mybir
from concourse.bass_utils import run_bass_kernel_spmd

BF16 = mybir.dt.bfloat16
F32 = mybir.dt.float32
AF = mybir.ActivationFunctionType
NPBF = ml_dtypes.bfloat16

N = 2048          # sequence length
EMB = 1024        # model dim
HL = 4            # heads per core
D = 64            # head dim
EC = HL * D       # head dims per core (256)
KT = N // 128     # 16 k-token tiles
QW = 1024         # q half width
NQH = N // QW     # 2 q halves
SCALE = 1.0 / np.sqrt(D)

_CACHE = {}


def build_program():
    nc = bacc.Bacc("TRN2", target_bir_lowering=False, debug=False, num_devices=8)

    xq_d = nc.declare_dram_parameter("xqT", [EMB, N], BF16, isOutput=False)
    xk_d = nc.declare_dram_parameter("xkT", [EMB, N], BF16, isOutput=False)
    xv_d = nc.declare_dram_parameter("xvT", [EMB, N], BF16, isOutput=False)
    wq_d = nc.declare_dram_parameter("wq", [EMB, EC], BF16, isOutput=False)
    wk_d = nc.declare_dram_parameter("wk", [EMB, EC], BF16, isOutput=False)
    wv_d = nc.declare_dram_parameter("wv", [EMB, EC], BF16, isOutput=False)
    wo_d = nc.declare_dram_parameter("wo", [EC, EMB], BF16, isOutput=False)
    bq_d = nc.declare_dram_parameter("bq", [1, EC], BF16, isOutput=False)
    bk_d = nc.declare_dram_parameter("bk", [1, EC], BF16, isOutput=False)
    bv_d = nc.declare_dram_parameter("bv", [1, EC], BF16, isOutput=False)
    out_d = nc.declare_dram_parameter("outT", [EMB, N], BF16, isOutput=True)
    twa_d = nc.declare_dram_parameter("twa", [N, N], BF16, isOutput=True)
    twb_d = nc.declare_dram_parameter("twb", [N, N], BF16, isOutput=True)

    with ExitStack() as ctx:
        tc = ctx.enter_context(tile.TileContext(nc))

        const = ctx.enter_context(tc.tile_pool(name="const", bufs=1))
        ones_bf = const.tile([1, 512], BF16, name="ones_bf")
        nc.vector.memset(ones_bf[:], 1.0)
        bq_sb = const.tile([1, EC], BF16, name="bq_sb")
        bk_sb = const.tile([1, EC], BF16, name="bk_sb")
        bv_sb = const.tile([1, EC], BF16, name="bv_sb")
        nc.sync.dma_start(bq_sb[:], bq_d[:])
        nc.sync.dma_start(bk_sb[:], bk_d[:])
        nc.sync.dma_start(bv_sb[:], bv_d[:])

        wpool = ctx.enter_context(tc.tile_pool(name="wpool", bufs=1))
        # W* stored k-chunk-major: [128, kc, EC]
        wq_sb = wpool.tile([128, 8, EC], BF16, name="wq_sb")
        wk_sb = wpool.tile([128, 8, EC], BF16, name="wk_sb")
        wv_sb = wpool.tile([128, 8, EC], BF16, name="wv_sb")
        wo_sb = wpool.tile([128, 2, EMB], BF16, name="wo_sb")
        for kc in range(8):
            nc.sync.dma_start(wk_sb[:, kc, :], wk_d[kc * 128:(kc + 1) * 128, :])
            nc.sync.dma_start(wq_sb[:, kc, :], wq_d[kc * 128:(kc + 1) * 128, :])
            nc.sync.dma_start(wv_sb[:, kc, :], wv_d[kc * 128:(kc + 1) * 128, :])
        for hc in range(2):
            nc.sync.dma_start(wo_sb[:, hc, :], wo_d[hc * 128:(hc + 1) * 128, :])

        qkv = ctx.enter_context(tc.tile_pool(name="qkv", bufs=1))
        qhT = qkv.tile([128, 2, N], BF16, name="qhT")    # [dim-half][128, tok]
        khT = qkv.tile([128, 2, N], BF16, name="khT")
        vh = qkv.tile([128, KT, HL, 65], BF16, name="vh")  # [tok128][kt][h][64 dims + ones]
        an = qkv.tile([128, 2, N], BF16, name="an")      # A_norm^T packed [hd-chunk][q]
        nc.vector.memset(vh[:, :, :, 64:65], 1.0)

        # ---------------- phase A: projections ----------------
        with tc.tile_pool(name="xpool", bufs=24) as xpool, \
             tc.tile_pool(name="prps", bufs=4, space="PSUM") as prps:

            def proj_qk(x_d, w_sb, b_sb, dst):
                xt = []
                for kc in range(8):
                    xtile = xpool.tile([128, N], BF16, name=f"xt{kc}", tag="x")
                    nc.sync.dma_start(xtile[:], x_d[kc * 128:(kc + 1) * 128, :])
                    xt.append(xtile)
                for dh in range(2):
                    for q4 in range(4):
                        ps = prps.tile([128, 512], F32, name="ps", tag="prps")
                        for kc in range(8):
                            nc.tensor.matmul(
                                ps[:],
                                w_sb[:, kc, dh * 128:(dh + 1) * 128],
                                xt[kc][:, q4 * 512:(q4 + 1) * 512],
                                start=(kc == 0), stop=False)
                        nc.tensor.matmul(
                            ps[:], b_sb[0:1, dh * 128:(dh + 1) * 128],
                            ones_bf[0:1, :], start=False, stop=True)
                        nc.scalar.copy(dst[:, dh, q4 * 512:(q4 + 1) * 512], ps[:])

            proj_qk(xk_d, wk_sb, bk_sb, khT)
            proj_qk(xq_d, wq_sb, bq_sb, qhT)

            xt = []
            for kc in range(8):
                xtile = xpool.tile([128, N], BF16, name=f"xv{kc}", tag="x")
                nc.sync.dma_start(xtile[:], xv_d[kc * 128:(kc + 1) * 128, :])
                xt.append(xtile)
            for t in range(KT):
                ps = prps.tile([128, EC], F32, name="psv", tag="prps")
                for kc in range(8):
                    nc.tensor.matmul(
                        ps[:], xt[kc][:, t * 128:(t + 1) * 128],
                        wv_sb[:, kc, :], start=(kc == 0), stop=False)
                nc.tensor.matmul(ps[:], ones_bf[0:1, 0:128], bv_sb[0:1, :],
                                 start=False, stop=True)
                nc.scalar.copy(vh[:, t, :, 0:64],
                               ps[:].rearrange("p (h c) -> p h c", h=HL))

        # ---------------- phase B: attention ----------------
        with tc.tile_pool(name="scps", bufs=2, space="PSUM") as scps, \
             tc.tile_pool(name="avps", bufs=1, space="PSUM") as avps, \
             tc.tile_pool(name="epool", bufs=36) as epool, \
             tc.tile_pool(name="rbp", bufs=4) as rbp, \
             tc.tile_pool(name="twp", bufs=6) as twp, \
             tc.tile_pool(name="smol", bufs=4) as smol:
            for hp in range(2):
                hA, hB = 2 * hp, 2 * hp + 1
                tw_d = twa_d if hp == 0 else twb_d
                for qc in range(NQH):
                    q0 = qc * QW
                    avA = avps.tile([65, QW], F32, name="avA", tag="avA")
                    avB = avps.tile([65, QW], F32, name="avB", tag="avB")
                    EA, EB = [], []
                    for kt in range(KT):
                        psA = scps.tile([128, QW], F32, name="psA", tag="ps")
                        psB = scps.tile([128, QW], F32, name="psB", tag="ps")
                        kcols = slice(kt * 128, (kt + 1) * 128)
                        for nx in (0, 512):
                            qs = slice(q0 + nx, q0 + nx + 512)
                            nc.tensor.matmul(psA[:, nx:nx + 512],
                                             khT[0:64, hp, kcols],
                                             qhT[0:64, hp, qs],
                                             start=True, stop=True)
                            nc.tensor.matmul(psB[:, nx:nx + 512],
                                             khT[64:128, hp, kcols],
                                             qhT[64:128, hp, qs],
                                             start=True, stop=True)
                        eA = epool.tile([128, QW], BF16, name="eA", tag="E")
                        eB = epool.tile([128, QW], BF16, name="eB", tag="E")
                        nc.scalar.activation(eA[:], psA[:], AF.Exp, scale=float(SCALE))
                        nc.scalar.activation(eB[:], psB[:], AF.Exp, scale=float(SCALE))
                        EA.append(eA)
                        EB.append(eB)
                        for nx in (0, 512):
                            nc.tensor.matmul(avA[:, nx:nx + 512], vh[:, kt, hA, :],
                                             eA[:, nx:nx + 512],
                                             start=(kt == 0), stop=(kt == KT - 1))
                            nc.tensor.matmul(avB[:, nx:nx + 512], vh[:, kt, hB, :],
                                             eB[:, nx:nx + 512],
                                             start=(kt == 0), stop=(kt == KT - 1))
                    # --- softmax denominators: 1/rowsum via pack -> DVE recip ---
                    # (DMA cannot read PSUM: stage the rowsum rows in SBUF first)
                    rsA = smol.tile([1, QW], F32, name="rsA", tag="rsA")
                    rsB = smol.tile([1, QW], F32, name="rsB", tag="rsB")
                    nc.scalar.copy(rsA[:], avA[64:65, :])
                    nc.scalar.copy(rsB[:], avB[64:65, :])
                    rsp = smol.tile([128, 16], F32, name="rsp", tag="rsp")
                    nc.sync.dma_start(rsp[0:64, :], rsA[:])
                    nc.sync.dma_start(rsp[64:128, :], rsB[:])
                    rcp = smol.tile([128, 16], F32, name="rcp", tag="rcp")
                    nc.vector.reciprocal(rcp[:], rsp[:])
                    rcb = smol.tile([128, 16], BF16, name="rcb", tag="rcb")
                    nc.vector.tensor_copy(rcb[:], rcp[:])
                    rA = smol.tile([1, QW], BF16, name="rA", tag="rA")
                    rB = smol.tile([1, QW], BF16, name="rB", tag="rB")
                    nc.sync.dma_start(rA[:], rcb[0:64, :])
                    nc.sync.dma_start(rB[:], rcb[64:128, :])
                    # broadcast 1/rowsum to 128 partitions via K=1 matmul
                    rbAp = scps.tile([128, QW], F32, name="rbAp", tag="ps")
                    rbBp = scps.tile([128, QW], F32, name="rbBp", tag="ps")
                    for nx in (0, 512):
                        nc.tensor.matmul(rbAp[:, nx:nx + 512], ones_bf[0:1, 0:128],
                                         rA[0:1, nx:nx + 512], start=True, stop=True)
                        nc.tensor.matmul(rbBp[:, nx:nx + 512], ones_bf[0:1, 0:128],
                                         rB[0:1, nx:nx + 512], start=True, stop=True)
                    rbA = rbp.tile([128, QW], BF16, name="rbA", tag="rb")
                    rbB = rbp.tile([128, QW], BF16, name="rbB", tag="rb")
                    nc.vector.tensor_copy(rbA[:], rbAp[:])
                    nc.vector.tensor_copy(rbB[:], rbBp[:])
                    # --- normalized attention outputs (A_norm^T) ---
                    nc.vector.tensor_mul(an[0:64, hp, q0:q0 + QW],
                                         avA[0:64, :], rbA[0:64, :])
                    tmpB = smol.tile([64, QW], BF16, name="tmpB", tag="tmpB")
                    nc.vector.tensor_mul(tmpB[:], avB[0:64, :], rbB[0:64, :])
                    nc.sync.dma_start(an[64:128, hp, q0:q0 + QW], tmpB[:])
                    # --- tw partial: sum of this head-pair's prob maps ---
                    for kt in range(KT):
                        twt = twp.tile([128, QW], BF16, name="twt", tag="tw")
                        tw2 = twp.tile([128, QW], BF16, name="tw2", tag="tw")
                        nc.vector.tensor_mul(twt[:], EA[kt][:], rbA[:])
                        nc.vector.tensor_mul(tw2[:], EB[kt][:], rbB[:])
                        nc.vector.tensor_add(twt[:], twt[:], tw2[:])
                        nc.sync.dma_start(
                            tw_d[kt * 128:(kt + 1) * 128, q0:q0 + QW], twt[:])

        # ---------------- phase C: output projection ----------------
        with tc.tile_pool(name="opps", bufs=4, space="PSUM") as opps, \
             tc.tile_pool(name="osb", bufs=4) as osb:
            for m in range(8):
                for qn in range(4):
                    ps = opps.tile([128, 512], F32, name="op", tag="op")
                    for hc in range(2):
                        nc.tensor.matmul(ps[:], wo_sb[:, hc, m * 128:(m + 1) * 128],
                                         an[:, hc, qn * 512:(qn + 1) * 512],
                                         start=(hc == 0), stop=(hc == 1))
                    ot = osb.tile([128, 512], BF16, name="ot", tag="ot")
                    if (m + qn) % 2:
                        nc.scalar.copy(ot[:], ps[:])
                    else:
                        nc.vector.tensor_copy(ot[:], ps[:])
                    nc.sync.dma_start(
                        out_d[m * 128:(m + 1) * 128, qn * 512:(qn + 1) * 512], ot[:])

    nc.compile()
    return nc


def _get_program():
    if "nc" not in _CACHE:
        _CACHE["nc"] = build_program()
    return _CACHE["nc"]


def _bf(x):
    return np.ascontiguousarray(np.asarray(x, dtype=np.float32)).astype(NPBF)


def make_in_maps(q, k, v, Wq, bq, Wk, bk, Wv, bv, Wo):
    in_maps = []
    for c in range(8):
        b, hg = c // 4, c % 4
        sl = slice(hg * EC, (hg + 1) * EC)
        in_maps.append({
            "xqT": _bf(q[b].T), "xkT": _bf(k[b].T), "xvT": _bf(v[b].T),
            "wq": _bf(Wq[:, sl]), "wk": _bf(Wk[:, sl]), "wv": _bf(Wv[:, sl]),
            "wo": _bf(Wo[sl, :]),
            "bq": _bf(bq[sl].reshape(1, EC)),
            "bk": _bf(bk[sl].reshape(1, EC)),
            "bv": _bf(bv[sl].reshape(1, EC)),
        })
    return in_maps


def gather_results(results, bo):
    out = np.zeros((2, N, EMB), np.float32)
    tw = np.zeros((N, N), np.float32)
    for c in range(8):
        b = c // 4
        out[b] += results[c]["outT"].astype(np.float32).T
        if b == 0:
            tw += results[c]["twa"].astype(np.float32)
            tw += results[c]["twb"].astype(np.float32)
    out += np.asarray(bo, dtype=np.float32).reshape(1, 1, EMB)
    return out, np.ascontiguousarray(tw.T)


def kernel(q, k, v, Wq, bq, Wk, bk, Wv, bv, Wo, bo, **run_kwargs):
    nc = _get_program()
    in_maps = make_in_maps(q, k, v, Wq, bq, Wk, bk, Wv, bv, Wo)
    res = run_bass_kernel_spmd(nc, in_maps, list(range(8)), **run_kwargs)
    out, tw = gather_results(res.results, bo)
    if run_kwargs:
        return (out, tw), res
    return (out, tw)
